# revision 27
# baseline (speedup 1.0000x reference)
"""EM capsule-routing (BaseCaps) Trainium2 Bass kernel.

Data-parallel over batch: 8 cores x (B=2, S=36) shard. Per core:
  - 72 (b,s) routing groups, I=288 in-caps, O=32 out-caps, P=16 pose dims.
  - votes cast to bf16, SBUF-resident in (p,o)-major free layout,
    packed as 18 supergroups x 4 bs x 9 [128,512] tiles (i-interleaved:
    tiles 2k,2k+1 = bs k rows 0..255, tile 8 = 4 x 32 leftover rows).
  - M-step contractions over I run on the TensorEngine with 0/1 selector
    (or in_act-valued) stationary operands; E-step quadratics run on
    DVE/ACT; per-(b,s) stat vectors are broadcast across partitions by
    GPSIMD / K=1 matmuls.
"""

import sys

sys.path.insert(0, "/opt/trn_rl_repo")

import numpy as np

import concourse.bass as bass
import concourse.mybir as mybir
from concourse import tile
from concourse import library_config


WAIT_SPLIT_COUNT = 0

f32 = mybir.dt.float32
bf16 = mybir.dt.float16  # fp16: votes ~N(0,1) fit easily; 8x finer mantissa than bf16
Alu = mybir.AluOpType
Act = mybir.ActivationFunctionType

NCORES = 8
B, S, I, O, P = 16, 36, 288, 32, 16
OP = O * P  # 512
BSH = B // NCORES  # 2
NBS = BSH * S  # 72
NSG = NBS // 4  # 18 supergroups of 4 bs
NT = NSG * 9  # 162 global tiles of 128 rows
EPS = 1e-7
ROUTINGS = 3


def _lam(i):
    return 0.01 * (1.0 - 0.95**i)


def build_nc(split_waits=True):
    nc = bass.Bass("TRN2", target_bir_lowering=False, debug=False, num_devices=NCORES)

    vd = nc.dram_tensor("votes", [BSH, S, I, O, P], f32, kind="ExternalInput").ap()
    iad = nc.dram_tensor("in_act", [BSH, S, I, 1, 1], f32, kind="ExternalInput").ap()
    bad = nc.dram_tensor("beta_a", [O], f32, kind="ExternalInput").ap()
    bvd = nc.dram_tensor("beta_v", [O], f32, kind="ExternalInput").ap()
    oad = nc.dram_tensor("out_act", [BSH, S, 1, O, 1], f32, kind="ExternalOutput").ap()
    md = nc.dram_tensor("means", [BSH, S, 1, O, P], f32, kind="ExternalOutput").ap()

    vflat = vd.rearrange("b s i o p -> (b s) i (o p)")  # [72, 288, 512]
    iaflat = iad.rearrange("b s i o p -> (b s) (i o p)")  # [72, 288]

    def dbc(row_ap, nrows, ncols):
        # [1, ncols] stats row -> DMA-broadcast source view [1, nrows, ncols]
        return row_ap.rearrange("a (x n) -> a x n", x=1).to_broadcast(
            (1, nrows, ncols)
        )

    with tile.TileContext(nc) as tc:
        with (
            tc.tile_pool(name="pers", bufs=1) as pers,
            tc.tile_pool(name="stats", bufs=1) as pst,
            tc.tile_pool(name="psS", bufs=2, space=bass.MemorySpace.PSUM) as psS,
        ):
            # ---------------- persistent tensors ----------------
            vbf = pers.tile([128, NT * 512], bf16)  # 162 KB/part
            ia_t = pers.tile([128, NT], f32)  # per-tile in_act columns
            ind8_f = pers.tile([128, 4], f32)  # tile-8 indicator fp32
            ind8_b = pers.tile([128, 4], bf16)
            colk_b = pers.tile([128, 16], bf16)  # 4 x [128,4] col-k indicators
            colk_f = pers.tile([128, 16], f32)
            ba_b = pers.tile([NBS, O], f32)
            bv_b = pers.tile([NBS, O], f32)
            base = pers.tile([NBS, O], f32)  # 16*bv - ba

            # stats (all fp32; (p,o) free order for [*,512])
            Sv_s = pst.tile([NBS, OP], f32)
            Svv_s = pst.tile([NBS, OP], f32)
            m_s = pst.tile([NBS, OP], f32)
            var_s = pst.tile([NBS, OP], f32)
            A_s = pst.tile([NBS, OP], f32)
            u1_s = pst.tile([NBS, OP], f32)
            m_bf = pst.tile([NBS, OP], bf16)
            A_bf = pst.tile([NBS, OP], bf16)
            c0_bf = pst.tile([NBS, O], bf16)
            rs_s = pst.tile([NBS, O], f32)
            R_s = pst.tile([NBS, O], f32)
            L_s = pst.tile([NBS, O], f32)
            t_s = pst.tile([NBS, O], f32)
            oa_s = pst.tile([NBS, O], f32)
            c0_s = pst.tile([NBS, O], f32)
            rs0_s = pst.tile([NBS, 1], f32)

            # ---------------- constants ----------------
            nc.vector.memset(ind8_f[:, :], 0.0)
            nc.vector.memset(colk_b[:, :], 0.0)
            nc.vector.memset(colk_f[:, :], 0.0)
            for k in range(4):
                nc.vector.memset(ind8_f[32 * k : 32 * k + 32, k : k + 1], 1.0)
                nc.vector.memset(colk_b[:, 4 * k + k : 4 * k + k + 1], 1.0)
                nc.vector.memset(colk_f[:, 4 * k + k : 4 * k + k + 1], 1.0)
            nc.vector.tensor_copy(ind8_b[:, :], ind8_f[:, :])

            # beta broadcast to [72, 32]
            bsmall = pers.tile([1, 2 * O], f32)
            nc.sync.dma_start(bsmall[0:1, 0:O], bad.rearrange("(a o) -> a o", a=1))
            nc.sync.dma_start(bsmall[0:1, O : 2 * O], bvd.rearrange("(a o) -> a o", a=1))
            nc.sync.dma_start(ba_b[:, :], dbc(bsmall[0:1, 0:O], NBS, O))
            nc.sync.dma_start(bv_b[:, :], dbc(bsmall[0:1, O : 2 * O], NBS, O))
            nc.vector.scalar_tensor_tensor(
                base[:, :], bv_b[:, :], 16.0, ba_b[:, :], Alu.mult, Alu.subtract
            )

            # =========== load + cast + iter-0 M-step ===========
            pstage = tc.alloc_tile_pool(name="stage", bufs=2)
            pt8 = tc.alloc_tile_pool(name="t8st", bufs=2)
            pld = tc.alloc_tile_pool(name="ld", bufs=1)
            # in_act bs-major copy (for rs0)
            ia_bs = pld.tile([NBS, I], f32)
            nc.sync.dma_start(ia_bs[:, :], iaflat)
            nc.vector.tensor_reduce(
                rs0_s[:, :], ia_bs[:, :], mybir.AxisListType.X, Alu.add
            )
            nc.vector.tensor_scalar_mul(rs0_s[:, :], rs0_s[:, :], 1.0 / O)

            for g in range(NSG):
                st8 = pt8.tile([128, 512], f32, tag="t8")
                sq8 = pt8.tile([128, 512], f32, tag="t8q")
                pSv = psS.tile([4, 512], f32, tag="sv")
                pSvv = psS.tile([4, 512], f32, tag="svv")
                for k in range(4):
                    bs = 4 * g + k
                    gt0 = 9 * g + 2 * k
                    # in_act columns
                    nc.sync.dma_start(
                        ia_t[:, gt0 : gt0 + 1],
                        iaflat[bs, 0:128].rearrange("(i a) -> i a", a=1),
                    )
                    nc.sync.dma_start(
                        ia_t[:, gt0 + 1 : gt0 + 2],
                        iaflat[bs, 128:256].rearrange("(i a) -> i a", a=1),
                    )
                    nc.sync.dma_start(
                        ia_t[32 * k : 32 * k + 32, 9 * g + 8 : 9 * g + 9],
                        iaflat[bs, 256:288].rearrange("(i a) -> i a", a=1),
                    )
                    for seg in range(2):
                        gt = gt0 + seg
                        stg = pstage.tile([128, 512], f32, tag="stg")
                        sq = pstage.tile([128, 512], f32, tag="sq")
                        nc.sync.dma_start(
                            stg[:, :],
                            vflat[bs, 128 * seg : 128 * seg + 128, :],
                        )
                        nc.scalar.activation(sq[:, :], stg[:, :], Act.Square)
                        # iter-0 selector: col k = in_act values ((o,p) col order)
                        sel0 = pstage.tile([128, 4], f32, tag="sel0")
                        nc.vector.tensor_scalar_mul(
                            sel0[:, :],
                            colk_f[:, 4 * k : 4 * k + 4],
                            ia_t[:, gt : gt + 1],
                        )
                        first = k == 0 and seg == 0
                        nc.tensor.matmul(
                            pSv[0:4, :], sel0[:, :], stg[:, :],
                            start=first, stop=False,
                        )
                        nc.tensor.matmul(
                            pSvv[0:4, :], sel0[:, :], sq[:, :],
                            start=first, stop=False,
                        )
                        # cast + (o,p)->(p,o) reorder into resident vbf
                        nc.vector.tensor_copy(
                            vbf[:, 512 * gt : 512 * (gt + 1)].rearrange(
                                "r (p o) -> r p o", p=P
                            ),
                            stg[:, :].rearrange("r (o p) -> r p o", p=P),
                        )
                    # leftover rows into tile-8 staging
                    nc.sync.dma_start(
                        st8[32 * k : 32 * k + 32, :], vflat[bs, 256:288, :]
                    )
                # tile 8 of supergroup g
                gt8 = 9 * g + 8
                sel08 = pstage.tile([128, 4], f32, tag="sel0")
                nc.vector.tensor_scalar_mul(
                    sel08[:, :], ind8_f[:, :], ia_t[:, gt8 : gt8 + 1]
                )
                nc.scalar.activation(sq8[:, :], st8[:, :], Act.Square)
                nc.tensor.matmul(
                    pSv[0:4, :], sel08[:, :], st8[:, :],
                    start=False, stop=True,
                )
                nc.tensor.matmul(
                    pSvv[0:4, :], sel08[:, :], sq8[:, :],
                    start=False, stop=True,
                )
                nc.vector.tensor_copy(
                    vbf[:, 512 * gt8 : 512 * (gt8 + 1)].rearrange(
                        "r (p o) -> r p o", p=P
                    ),
                    st8[:, :].rearrange("r (o p) -> r p o", p=P),
                )
                # evacuate supergroup stats raw ((o,p) order):
                # DVE to base-0 staging, then SBUF->SBUF DMA to stats rows
                ev = pt8.tile([4, 1024], f32, tag="ev", bufs=2)
                nc.vector.tensor_copy(ev[0:4, 0:512], pSv[0:4, :])
                nc.vector.tensor_copy(ev[0:4, 512:1024], pSvv[0:4, :])
                nc.sync.dma_start(u1_s[4 * g : 4 * g + 4, :], ev[0:4, 0:512])
                nc.sync.dma_start(var_s[4 * g : 4 * g + 4, :], ev[0:4, 512:1024])

            pld.release()
            pt8.release()
            pstage.release()
            pwork = tc.alloc_tile_pool(name="work", bufs=2)
            psg = tc.alloc_tile_pool(name="sgw", bufs=2)

            # =========== routing iterations ===========
            for it in range(ROUTINGS):
                # ---- stats phase (batched [72, *] fp32) ----
                if it == 0:
                    # reorder raw (o,p) stats into (p,o); rr0 = 1/32 uniform
                    nc.vector.tensor_copy(
                        Sv_s[:, :].rearrange("b (p o) -> b p o", p=P),
                        u1_s[:, :].rearrange("b (o p) -> b p o", p=P),
                    )
                    nc.vector.tensor_copy(
                        Svv_s[:, :].rearrange("b (p o) -> b p o", p=P),
                        var_s[:, :].rearrange("b (o p) -> b p o", p=P),
                    )
                    nc.vector.tensor_scalar_mul(Sv_s[:, :], Sv_s[:, :], 1.0 / O)
                    nc.vector.tensor_scalar_mul(Svv_s[:, :], Svv_s[:, :], 1.0 / O)
                    nc.vector.tensor_copy(
                        rs_s[:, :],
                        rs0_s[:, 0:1].to_broadcast((NBS, O)),
                    )

                # R = 1/(rs+eps)
                nc.vector.tensor_scalar_add(R_s[:, :], rs_s[:, :], EPS)
                nc.vector.reciprocal(R_s[:, :], R_s[:, :])
                # broadcast helpers over p: AP dims (p step0, o step1)
                def bc_po(x):
                    # [72, O] -> [72, P, O] with p step 0
                    return x[:, :].rearrange("b (x o) -> b x o", x=1).to_broadcast(
                        (NBS, P, O)
                    )

                m3 = m_s[:, :].rearrange("b (p o) -> b p o", p=P)
                # m = Sv * R
                nc.vector.tensor_tensor(m3, Sv_s[:, :].rearrange(
                    "b (p o) -> b p o", p=P), bc_po(R_s), Alu.mult)
                # var = (Svv - 2 m Sv + m^2 rs) * R
                u13 = u1_s[:, :].rearrange("b (p o) -> b p o", p=P)
                nc.vector.tensor_tensor(
                    u13, m3, Sv_s[:, :].rearrange("b (p o) -> b p o", p=P), Alu.mult
                )
                nc.vector.scalar_tensor_tensor(
                    u1_s[:, :], u1_s[:, :], -2.0, Svv_s[:, :], Alu.mult, Alu.add
                )
                v3 = var_s[:, :].rearrange("b (p o) -> b p o", p=P)
                nc.vector.tensor_tensor(v3, m3, m3, Alu.mult)
                nc.vector.tensor_tensor(v3, v3, bc_po(rs_s), Alu.mult)
                nc.vector.tensor_tensor(v3, v3, u1_s[:, :].rearrange(
                    "b (p o) -> b p o", p=P), Alu.add)
                nc.vector.tensor_tensor(v3, v3, bc_po(R_s), Alu.mult)
                # A = 1/(2 var + eps)
                nc.vector.tensor_scalar(
                    A_s[:, :], var_s[:, :], 2.0, EPS, Alu.mult, Alu.add
                )
                nc.vector.reciprocal(A_s[:, :], A_s[:, :])
                # L = sum_p ln(var)  (2*sum_p log std)
                nc.vector.tensor_scalar_max(var_s[:, :], var_s[:, :], 1e-30)
                nc.scalar.activation(u1_s[:, :], var_s[:, :], Act.Ln)
                nc.vector.tensor_reduce(
                    L_s[:, :],
                    u1_s[:, :].rearrange("b (p o) -> b o p", p=P),
                    mybir.AxisListType.X,
                    Alu.add,
                )
                # out_act = sigmoid(lam*(ba - 16 bv - 0.5 rs*L))
                #   e_arg = lam*(16 bv - ba) + 0.5*lam*rs*L ; oa = 1/(1+exp(e_arg))
                lam = _lam(it)
                nc.vector.tensor_tensor(t_s[:, :], rs_s[:, :], L_s[:, :], Alu.mult)
                nc.vector.scalar_tensor_tensor(
                    t_s[:, :], t_s[:, :], 0.5, base[:, :], Alu.mult, Alu.add
                )
                nc.vector.tensor_scalar_mul(t_s[:, :], t_s[:, :], lam)
                nc.scalar.activation(t_s[:, :], t_s[:, :], Act.Exp)
                nc.vector.tensor_scalar_add(t_s[:, :], t_s[:, :], 1.0)
                nc.vector.reciprocal(oa_s[:, :], t_s[:, :])

                if it == ROUTINGS - 1:
                    # final outputs: out_act, means
                    m_op = u1_s
                    nc.vector.tensor_copy(
                        m_op[:, :].rearrange("b (o p) -> b o p", p=P),
                        m_s[:, :].rearrange("b (p o) -> b o p", p=P),
                    )
                    nc.sync.dma_start(
                        oad.rearrange("b s x o y -> (b s) (x o y)"), oa_s[:, :]
                    )
                    nc.sync.dma_start(
                        md.rearrange("b s x o p -> (b s) (x o p)"), m_op[:, :]
                    )
                    break

                # c0 = ln(oa+eps) - 0.5 L ; m_bf for broadcasts
                nc.vector.tensor_scalar_add(t_s[:, :], oa_s[:, :], EPS)
                nc.scalar.activation(t_s[:, :], t_s[:, :], Act.Ln)
                nc.vector.scalar_tensor_tensor(
                    c0_s[:, 0:O], L_s[:, :], -0.5, t_s[:, :], Alu.mult, Alu.add
                )
                nc.vector.tensor_copy(m_bf[:, :], m_s[:, :])
                nc.vector.tensor_copy(A_bf[:, :], A_s[:, :])
                nc.vector.tensor_copy(c0_bf[:, :], c0_s[:, :])

                # ---- fused E(it) + M(it+1) ----
                for g in range(NSG):
                    sgbase = 9 * g * 512
                    q_sg = psg.tile([128, 288], f32, tag="q", bufs=1)
                    zz_sg = psg.tile([128, 288], f32, tag="zz")
                    e_sg = psg.tile([128, 288], f32, tag="zz", bufs=2)
                    rr_sg = psg.tile([128, 288], bf16, tag="rr")
                    s9 = psg.tile([128, 9], f32, tag="s9")
                    w9 = psg.tile([128, 9], f32, tag="w9")
                    c0e = psg.tile([128, 288], bf16, tag="c0e")
                    pSv = psS.tile([4, 512], f32, tag="sv")
                    pSvv = psS.tile([4, 512], f32, tag="svv")
                    pAux = psS.tile([4, 512], f32, tag="aux")
                    # tile-8 m/A/c0 via 32-channel partition broadcasts
                    m8 = pwork.tile([128, 512], bf16, tag="m8", bufs=1)
                    a8 = pwork.tile([128, 512], bf16, tag="a8", bufs=1)
                    for k in range(4):
                        bs = 4 * g + k
                        nc.sync.dma_start(
                            m8[32 * k : 32 * k + 32, :],
                            dbc(m_bf[bs : bs + 1, :], 32, OP),
                        )
                        nc.sync.dma_start(
                            a8[32 * k : 32 * k + 32, :],
                            dbc(A_bf[bs : bs + 1, :], 32, OP),
                        )
                        nc.sync.dma_start(
                            c0e[32 * k : 32 * k + 32, 256:288],
                            dbc(c0_bf[bs : bs + 1, :], 32, O),
                        )

                    for k in range(4):
                        bs = 4 * g + k
                        pair = vbf[
                            :, sgbase + 1024 * k : sgbase + 1024 * (k + 1)
                        ]
                        mexp = pwork.tile([128, 512], bf16, tag="mexp")
                        aexp = pwork.tile([128, 512], bf16, tag="aexp")
                        nc.sync.dma_start(
                            mexp[:, :], dbc(m_bf[bs : bs + 1, :], 128, OP)
                        )
                        nc.sync.dma_start(
                            aexp[:, :], dbc(A_bf[bs : bs + 1, :], 128, OP)
                        )
                        nc.sync.dma_start(
                            c0e[:, 64 * k : 64 * k + 32],
                            dbc(c0_bf[bs : bs + 1, :], 128, O),
                        )
                        nc.sync.dma_start(
                            c0e[:, 64 * k + 32 : 64 * k + 64],
                            dbc(c0_bf[bs : bs + 1, :], 128, O),
                        )
                        d = pwork.tile([128, 1024], bf16, tag="big", bufs=3)
                        dd = pwork.tile([128, 1024], bf16, tag="big", bufs=3)
                        f = pwork.tile([128, 1024], bf16, tag="big", bufs=3)
                        bca = lambda x: x[:, :].rearrange(
                            "r (x n) -> r x n", x=1
                        ).to_broadcast((128, 2, 512))
                        nc.vector.tensor_tensor(
                            d[:, :].rearrange("r (t n) -> r t n", t=2),
                            pair.rearrange("r (t n) -> r t n", t=2),
                            bca(mexp),
                            Alu.subtract,
                        )
                        nc.scalar.activation(dd[:, :], d[:, :], Act.Square)
                        nc.vector.tensor_tensor(
                            f[:, :].rearrange("r (t n) -> r t n", t=2),
                            dd[:, :].rearrange("r (t n) -> r t n", t=2),
                            bca(aexp),
                            Alu.mult,
                        )
                        # tree reduce over p (p-major halves are contiguous)
                        tr1 = pwork.tile([128, 512], bf16, tag="b8", bufs=3)
                        tr2 = pwork.tile([128, 256], bf16, tag="tr2", bufs=1)
                        tr3 = pwork.tile([128, 128], bf16, tag="tr3", bufs=1)
                        f4 = f[:, :].rearrange("r (t h n) -> r t h n", t=2, h=2)
                        nc.vector.tensor_tensor(
                            tr1[:, :].rearrange("r (t n) -> r t n", t=2),
                            f4[:, :, 0, :],
                            f4[:, :, 1, :],
                            Alu.add,
                        )
                        t14 = tr1[:, :].rearrange("r (t h n) -> r t h n", t=2, h=2)
                        nc.vector.tensor_tensor(
                            tr2[:, :].rearrange("r (t n) -> r t n", t=2),
                            t14[:, :, 0, :],
                            t14[:, :, 1, :],
                            Alu.add,
                        )
                        t24 = tr2[:, :].rearrange("r (t h n) -> r t h n", t=2, h=2)
                        nc.vector.tensor_tensor(
                            tr3[:, :].rearrange("r (t n) -> r t n", t=2),
                            t24[:, :, 0, :],
                            t24[:, :, 1, :],
                            Alu.add,
                        )
                        t34 = tr3[:, :].rearrange("r (t h n) -> r t h n", t=2, h=2)
                        nc.vector.tensor_tensor(
                            q_sg[:, 64 * k : 64 * (k + 1)].rearrange(
                                "r (t n) -> r t n", t=2
                            ),
                            t34[:, :, 0, :],
                            t34[:, :, 1, :],
                            Alu.add,
                        )
                    # tile 8 E-step
                    v8 = vbf[:, sgbase + 8 * 512 : sgbase + 9 * 512]
                    d8 = pwork.tile([128, 512], bf16, tag="b8", bufs=3)
                    dd8 = pwork.tile([128, 512], bf16, tag="b8", bufs=3)
                    f8 = pwork.tile([128, 512], bf16, tag="b8", bufs=3)
                    nc.vector.tensor_tensor(d8[:, :], v8, m8[:, :], Alu.subtract)
                    nc.scalar.activation(dd8[:, :], d8[:, :], Act.Square)
                    nc.vector.tensor_tensor(f8[:, :], dd8[:, :], a8[:, :], Alu.mult)
                    tr18 = pwork.tile([128, 256], bf16, tag="tr18", bufs=1)
                    tr28 = pwork.tile([128, 128], bf16, tag="tr28", bufs=1)
                    tr38 = pwork.tile([128, 64], bf16, tag="tr38", bufs=1)
                    f8h = f8[:, :].rearrange("r (h n) -> r h n", h=2)
                    nc.vector.tensor_tensor(tr18[:, :], f8h[:, 0, :], f8h[:, 1, :], Alu.add)
                    t1h = tr18[:, :].rearrange("r (h n) -> r h n", h=2)
                    nc.vector.tensor_tensor(tr28[:, :], t1h[:, 0, :], t1h[:, 1, :], Alu.add)
                    t2h = tr28[:, :].rearrange("r (h n) -> r h n", h=2)
                    nc.vector.tensor_tensor(tr38[:, :], t2h[:, 0, :], t2h[:, 1, :], Alu.add)
                    t3h = tr38[:, :].rearrange("r (h n) -> r h n", h=2)
                    nc.vector.tensor_tensor(
                        q_sg[:, 256:288], t3h[:, 0, :], t3h[:, 1, :], Alu.add
                    )
                    # zz = c0_exp - q ; softmax over o (batched, no max-sub)
                    nc.vector.tensor_tensor(
                        zz_sg[:, :], c0e[:, :], q_sg[:, :], Alu.subtract
                    )
                    nc.scalar.activation(e_sg[:, :], zz_sg[:, :], Act.Exp)
                    nc.vector.tensor_reduce(
                        s9[:, :],
                        e_sg[:, :].rearrange("r (t o) -> r t o", t=9),
                        mybir.AxisListType.X,
                        Alu.add,
                    )
                    nc.vector.reciprocal(s9[:, :], s9[:, :])
                    nc.vector.tensor_tensor(
                        w9[:, :], s9[:, :], ia_t[:, 9 * g : 9 * g + 9], Alu.mult
                    )
                    nc.vector.tensor_tensor(
                        rr_sg[:, :].rearrange("r (t o) -> r t o", t=9),
                        e_sg[:, :].rearrange("r (t o) -> r t o", t=9),
                        w9[:, :].rearrange("r (t x) -> r t x", x=1).to_broadcast(
                            (128, 9, O)
                        ),
                        Alu.mult,
                    )
                    # ---- M-step matmuls for iteration it+1 ----
                    for k in range(4):
                        bs = 4 * g + k
                        pair = vbf[
                            :, sgbase + 1024 * k : sgbase + 1024 * (k + 1)
                        ]
                        t1 = pwork.tile([128, 1024], bf16, tag="big", bufs=3)
                        t2 = pwork.tile([128, 1024], bf16, tag="big", bufs=3)
                        rrb = (
                            rr_sg[:, 64 * k : 64 * (k + 1)]
                            .rearrange("r (t x o) -> r t x o", t=2, x=1)
                            .to_broadcast((128, 2, P, O))
                        )
                        nc.vector.tensor_tensor(
                            t1[:, :].rearrange("r (t p o) -> r t p o", t=2, p=P),
                            pair.rearrange("r (t p o) -> r t p o", t=2, p=P),
                            rrb,
                            Alu.mult,
                        )
                        nc.vector.tensor_tensor(t2[:, :], t1[:, :], pair, Alu.mult)
                        ck = colk_b[:, 4 * k : 4 * k + 4]
                        for t in range(2):
                            first = k == 0 and t == 0
                            nc.tensor.matmul(
                                pSv[0:4, :],
                                ck,
                                t1[:, 512 * t : 512 * (t + 1)],
                                start=first,
                                stop=False,
                            )
                            nc.tensor.matmul(
                                pSvv[0:4, :],
                                ck,
                                t2[:, 512 * t : 512 * (t + 1)],
                                start=first,
                                stop=False,
                            )
                            nc.tensor.matmul(
                                pAux[0:4, 0:O],
                                ck,
                                rr_sg[:, 64 * k + 32 * t : 64 * k + 32 * (t + 1)],
                                start=first,
                                stop=False,
                            )
                    # tile-8 M-step
                    t18 = pwork.tile([128, 512], bf16, tag="b8", bufs=3)
                    t28 = pwork.tile([128, 512], bf16, tag="b8", bufs=3)
                    rrb8 = (
                        rr_sg[:, 256:288]
                        .rearrange("r (x o) -> r x o", x=1)
                        .to_broadcast((128, P, O))
                    )
                    nc.vector.tensor_tensor(
                        t18[:, :].rearrange("r (p o) -> r p o", p=P),
                        v8.rearrange("r (p o) -> r p o", p=P),
                        rrb8,
                        Alu.mult,
                    )
                    nc.vector.tensor_tensor(t28[:, :], t18[:, :], v8, Alu.mult)
                    nc.tensor.matmul(
                        pSv[0:4, :], ind8_b[:, :], t18[:, :],
                        start=False, stop=True,
                    )
                    nc.tensor.matmul(
                        pSvv[0:4, :], ind8_b[:, :], t28[:, :],
                        start=False, stop=True,
                    )
                    nc.tensor.matmul(
                        pAux[0:4, 0:O], ind8_b[:, :],
                        rr_sg[:, 256:288],
                        start=False, stop=True,
                    )
                    # evacuate supergroup stats (already (p,o) ordered)
                    ev = pwork.tile([4, 1056], f32, tag="ev", bufs=1)
                    nc.vector.tensor_copy(ev[0:4, 0:512], pSv[0:4, :])
                    nc.vector.tensor_copy(ev[0:4, 512:1024], pSvv[0:4, :])
                    nc.vector.tensor_copy(ev[0:4, 1024:1056], pAux[0:4, 0:O])
                    nc.sync.dma_start(Sv_s[4 * g : 4 * g + 4, :], ev[0:4, 0:512])
                    nc.sync.dma_start(Svv_s[4 * g : 4 * g + 4, :], ev[0:4, 512:1024])
                    nc.sync.dma_start(rs_s[4 * g : 4 * g + 4, :], ev[0:4, 1024:1056])
            psg.release()
            pwork.release()

    # ---- post-pass: the pinned walrus rejects >1 sync-wait per
    # instruction (CoreV2/V3 setupSyncWait "Too many sync wait commands").
    # Hoist extra waits onto same-engine Drain instructions just before. ----
    global WAIT_SPLIT_COUNT
    n_split = 0
    if not split_waits:
        return nc
    for fn in nc.m.functions:
        for bb in fn.blocks:
            out_list = []
            for inst in bb.instructions:
                si = inst.sync_info
                waits = list(si.on_wait) if (si is not None and si.on_wait) else []
                if len(waits) > 1:
                    for w in waits[:-1]:
                        d = mybir.InstDrain(name=f"wsplit-{n_split}")
                        n_split += 1
                        d.engine = inst.engine
                        d.sync_info = mybir.SyncInfo(on_wait=[w], on_update=[])
                        out_list.append(d)
                    inst.sync_info = mybir.SyncInfo(
                        on_wait=[waits[-1]], on_update=list(si.on_update or [])
                    )
                out_list.append(inst)
            if n_split:
                bb.instructions[:] = out_list
    WAIT_SPLIT_COUNT = n_split
    return nc


_NC = None


def _get_nc():
    global _NC
    if _NC is None:
        _NC = build_nc()
    return _NC


def _run(in_act, votes, beta_a, beta_v, trace=False, **trace_kwargs):
    nc = _get_nc()
    from concourse.bass_utils import run_bass_kernel_spmd

    votes = np.asarray(votes, dtype=np.float32)
    in_act = np.asarray(in_act, dtype=np.float32)
    beta_a = np.asarray(beta_a, dtype=np.float32)
    beta_v = np.asarray(beta_v, dtype=np.float32)

    in_maps = []
    for c in range(NCORES):
        sl = slice(c * BSH, (c + 1) * BSH)
        in_maps.append(
            {
                "votes": np.ascontiguousarray(votes[sl]),
                "in_act": np.ascontiguousarray(in_act[sl]),
                "beta_a": beta_a,
                "beta_v": beta_v,
            }
        )
    return run_bass_kernel_spmd(
        nc, in_maps, core_ids=list(range(NCORES)), trace=trace, **trace_kwargs
    )


def kernel(in_act, votes, beta_a, beta_v, routings):
    assert int(routings) == ROUTINGS
    res = _run(in_act, votes, beta_a, beta_v)
    oa = np.concatenate([r["out_act"] for r in res.results], axis=0)
    mm = np.concatenate([r["means"] for r in res.results], axis=0)
    return oa, mm
